# revision 59
# baseline (speedup 1.0000x reference)
"""Trainium2 Bass kernel for nn_MultiHeadAttention_8684423872640.

Math: the reference collapses algebraically. With
  s[m]   = Wfc[0, m // 64] / sqrt(64)
  A      = (Wk * s[:,None]).T @ Wq / L            # [1024, 1024] weights-only
  u      = Wk.T @ (s * bq)                        # [1024]
  qv     = Wq.T @ (s * bk) / L                    # [1024]
  c0     = (s * bk) @ bq + bfc[0]
the output for batch b is
  xsum_b = sum_l x[b, l, :]                       # [1024]
  w_eff  = A @ xsum_b + u                         # [1024]
  c      = qv @ xsum_b + c0
  out[b, l, 0] = x[b, l, :] @ w_eff + c

Sharding: data-parallel over B — core c handles batch c.

Schedule (fp8 default):
  - x ships as e4m3 (4 MiB/core) and A as e4m3 at a static pow2 scale
    (1 MiB): 5.3 MB total vs 10.5 MB for bf16 — the DMA-engine pool
    (~360 GB/s, serialized) is the floor, so bytes are wall-clock.
    Single SP HWDGE ring, A slices interleaved between x tiles, tail x
    tile in two halves to shorten the last dependency chain.
  - rowsums: each tile split ACT (activation-copy accum) / DVE
    (tensor_reduce) by engine rate; xsum lands ~1.5-2us after its tile.
  - folds: PSUM [128,8] accumulation opened by ONE identity@u matmul
    (start=True covering the whole region — per-column start flags
    lazily re-mark the coarse PSUM zero-region and wipe earlier
    accumulation), then A-block@xsum 1-col matmuls, ldweights hidden
    (~25ns each). Dummy 512-col matmuls pad the PE queue so the DVFS
    p-state stays at 2.4GHz (idle drops it to 1.2GHz for ~3us).
  - pass 2 flipped: x chunks are the STATIONARY ([128,128], loads
    hidden), w8=e4m3(w*2^13) the moving 1-col operand: 256 matmuls
    ~26ns each ≈ 6.7us, accumulating out as PSUM [128, 32] (l = ch*128
    + p), opened early by one zeroing matmul. One DVE epilogue
    (o + c*SW)/SW and one 16KB store.
"""

import os
import sys
import functools
import numpy as np

B, L, N = 8, 4096, 1024
D_K = 64
NCORES = 8
PT = N // 128   # 8 feature tiles
NCH = L // 128  # 32 pass-2 output chunks

SW = 2.0 ** 13          # w / qv / c scale so fp8(w*SW) is in normal range

_TRN_REPO = "/opt/trn_rl_repo"

MODE = os.environ.get("KERNEL_MODE", "fp8")   # "fp8" | "bf16"
# A-matrix extra scale (pow2): at8 = fp8(A * SA); w dequant folds SW/SA
# into the quantize-activation's scale. Chosen so fp8(A*SA) has ~8x
# headroom under the 448 e4m3 max.
SA_EXTRA = {"fp8": None, "bf16": 1.0}

DUM = {
    "fp8": [21, 11, 11, 22, 8, 7, 10, 12],
    "bf16": [24, 8, 13, 13, 13, 13, 13, 11],
}


def _ensure_path():
    if _TRN_REPO not in sys.path and os.path.isdir(_TRN_REPO):
        sys.path.insert(0, _TRN_REPO)


@functools.lru_cache(maxsize=2)
def _build(mode: str, sa_over_sw: float):
    """Build + compile the per-core Bass program. sa_over_sw = SA/SW."""
    _ensure_path()
    import concourse.bass as bass  # noqa: F401
    import concourse.tile as tile
    from concourse import bacc, mybir

    f32 = mybir.dt.float32
    bf16 = mybir.dt.bfloat16
    fp8 = mybir.dt.float8e4
    dtx = fp8 if mode == "fp8" else bf16

    nc = bacc.Bacc(
        "TRN2",
        target_bir_lowering=False,
        debug=False,
        enable_asserts=False,
        num_devices=NCORES,
    )

    xT = nc.dram_tensor("xT", [N, L], dtx, kind="ExternalInput").ap()
    atr = nc.dram_tensor("atr", [128, PT * N], dtx, kind="ExternalInput").ap()
    qv8 = nc.dram_tensor("qv8", [128, PT], dtx, kind="ExternalInput").ap()
    u8 = nc.dram_tensor("u8", [128, PT], bf16, kind="ExternalInput").ap()
    id8 = nc.dram_tensor("id8", [128, 128], bf16, kind="ExternalInput").ap()
    ones_r = nc.dram_tensor("ones_r", [1, 128], f32, kind="ExternalInput").ap()
    c0 = nc.dram_tensor("c0", [1, 1], f32, kind="ExternalInput").ap()
    out_d = nc.dram_tensor("out", [128, NCH], f32, kind="ExternalOutput").ap()

    with tile.TileContext(nc) as tc:
        with (
            tc.tile_pool(name="xpool", bufs=1) as xpool,
            tc.tile_pool(name="cpool", bufs=1) as cpool,
            tc.tile_pool(name="spool", bufs=2) as spool,
            tc.tile_pool(name="xsums", bufs=18) as xsums,
            tc.tile_pool(name="wps", bufs=1, space="PSUM") as wps,
            tc.tile_pool(name="cps", bufs=1, space="PSUM") as cps,
            tc.tile_pool(name="ops", bufs=1, space="PSUM") as ops,
            tc.tile_pool(name="dps", bufs=1, space="PSUM") as dps,
        ):
            # ---- small constants: memsets + DVE-ring DMAs first so the
            # PE's opening matmuls and dummies can start right after the
            # preamble (SWDGE issue is ~1us per DMA — too slow for these)
            dum = cpool.tile([128, 512], dtx, tag="dum")
            nc.gpsimd.memset(dum[:], 1.0)
            zz = cpool.tile([128, NCH], bf16, tag="zz")
            nc.gpsimd.memset(zz[:], 0.0)
            id_sb = cpool.tile([128, 128], bf16, tag="id")
            nc.scalar.dma_start(id_sb[:], id8[:])
            u_sb = cpool.tile([128, PT], bf16, tag="u")
            nc.scalar.dma_start(u_sb[:], u8[:])
            # qv/c0/ones ride the ACT ring too: SWDGE issues are ~1us
            # each and their packets land mid-x-stream in the shared DMA
            # pool; ACT's first rowsum is x0-gated until ~11us anyway,
            # and this frees gpsimd for the combine ops from the start.
            qv_sb = cpool.tile([128, PT], dtx, tag="qv")
            nc.scalar.dma_start(qv_sb[:], qv8[:])
            c0_sb = cpool.tile([1, 1], f32, tag="c0")
            nc.scalar.dma_start(c0_sb[:], c0[:])
            ones_sb = cpool.tile([1, 128], f32, tag="ones")
            nc.scalar.dma_start(ones_sb[:], ones_r[:])

            # ---- bulk stream: one SP HWDGE ring, explicit order ----
            xall = xpool.tile([128, PT * L], dtx, tag="x")
            at_sb = cpool.tile([128, PT * N], dtx, tag="at")

            xv3 = xall[:].rearrange("p (t l) -> p t l", t=PT)

            def dma_x(pt, lo=0, hi=L):
                nc.sync.dma_start(
                    xall[:, pt * L + lo: pt * L + hi],
                    xT[pt * 128:(pt + 1) * 128, lo:hi])

            def dma_x_pair(pt):
                # two row-tiles of xT in ONE dma (fewer SEQ issues: the
                # sync queue's ~0.65us/issue was outpacing the transfers);
                # src rearranged so dims pair 1:1 with the dst view
                src = xT[pt * 128:(pt + 2) * 128, :].rearrange(
                    "(t p) l -> p t l", t=2)
                nc.sync.dma_start(xv3[:, pt:pt + 2, :], src)

            def dma_at2(pt):
                nc.sync.dma_start(
                    at_sb[:, pt * N:(pt + 2) * N], atr[:, pt * N:(pt + 2) * N])

            dma_x(0)
            dma_x(1)
            dma_at2(0)
            dma_x(2)
            dma_x(3)
            dma_at2(2)
            dma_x(4)
            dma_x(5)
            dma_at2(4)
            dma_x(6)
            dma_at2(6)
            dma_x(7, 0, 3 * L // 4)
            dma_x(7, 3 * L // 4, L)

            # ---- rowsums: ACT / DVE / GPSIMD 3-lane split per tile ----
            # fp8 rates: ACT .97 ns/col (+278ns accum read), DVE 1.12
            # (2x needs 2-byte dtypes). GPSIMD can't X-reduce but CAN
            # pairwise-add fp8 halves into bf16 (~2 ns/out-col), which
            # DVE then reduces at its 2x bf16 rate — a third lane that
            # turns the 2-engine backlog into slack.
            act_scr = cpool.tile([128, 2304], dtx, tag="ascr")

            xm_mm = [None] * PT   # fold + c-path moving operand

            # lane budget: ACT .97 ns/col + 278ns accum read; DVE 1.12
            # ns/col (fp8 is 1x; 2x needs 2-byte dtypes) + combine. The
            # serial ACT+DVE work (~19us each for all 8 tiles) exceeds
            # the 15us stream, so GPSIMD takes tiles 2 and 5 entirely
            # (pairwise fp8 add at ~2.4 ns/out-col, slow but free
            # capacity) and DVE finishes them at its 2x bf16 rate.
            ACT_SHARE = (16, 32) if mode == "fp8" else (10, 32)
            gp_scr = cpool.tile([128, L], bf16, tag="gscr")
            partmap = {}

            def emit_act_dve(pt, lo, w, base):
                src = lambda a, b: xall[:, pt * L + lo + a: pt * L + lo + b]
                wa = w * ACT_SHARE[0] // ACT_SHARE[1] // 16 * 16
                parts = partmap[pt]
                nc.scalar.activation(
                    act_scr[:, 0:wa], src(0, wa),
                    mybir.ActivationFunctionType.Copy,
                    bias=0.0, accum_out=parts[:, base:base + 1])
                nc.vector.tensor_reduce(
                    parts[:, base + 1:base + 2], src(wa, w),
                    axis=mybir.AxisListType.X, op=mybir.AluOpType.add)

            def emit_gp_add(pt, slot):
                nc.gpsimd.tensor_add(
                    gp_scr[:, slot * (L // 2):slot * (L // 2) + L // 2],
                    xall[:, pt * L: pt * L + L // 2],
                    xall[:, pt * L + L // 2: (pt + 1) * L])

            def emit_gp_reduce(pt, slot):
                nc.vector.tensor_reduce(
                    partmap[pt][:, 0:1],
                    gp_scr[:, slot * (L // 2):slot * (L // 2) + L // 2],
                    axis=mybir.AxisListType.X, op=mybir.AluOpType.add)

            def emit_combine(pt, n):
                # fp8: combine + convert ride the otherwise-idle GPSIMD
                # (tiny [128,1] ops) so DVE's serial rowsum chain stays
                # pure reduces. xsum ships halved: device fp8e4 is e4m3
                # max 240 (not e4m3fn's 448) and |xsum| reaches ~430;
                # the w dequant and host-side qv scale compensate.
                parts = partmap[pt]
                if mode == "fp8":
                    xs = xsums.tile([128, 1], f32, tag="xs", name=f"xs{pt}")
                    nc.gpsimd.tensor_add(xs[:], parts[:, 0:1], parts[:, 1:2])
                    for j in range(2, n):
                        nc.gpsimd.tensor_add(xs[:], xs[:], parts[:, j:j + 1])
                    xm_mm[pt] = xsums.tile([128, 1], fp8, tag="x8", name=f"x8{pt}")
                    nc.gpsimd.tensor_scalar_mul(xm_mm[pt][:], xs[:], 0.5)
                else:
                    xs = xsums.tile([128, 1], f32, tag="xs", name=f"xs{pt}")
                    nc.vector.tensor_reduce(
                        xs[:], parts[:, 0:n], axis=mybir.AxisListType.X,
                        op=mybir.AluOpType.add)
                    xm_mm[pt] = xsums.tile([128, 1], bf16, tag="xb", name=f"xb{pt}")
                    nc.vector.tensor_copy(xm_mm[pt][:], xs[:])

            def alloc_parts(pt, n):
                partmap[pt] = xsums.tile([128, n], f32, tag="p", name=f"p{pt}")

            # ---- PE stream ----
            w_ps = wps.tile([128, PT], f32, tag="wps")
            c_ps = cps.tile([1, 1], f32, tag="cps")
            d_ps = dps.tile([1, 512], f32, tag="dps")
            # four separate quarter tiles: the per-quarter epilogue READS
            # must not alias the next quarter's matmul WRITES (coarse WAR
            # tracking stalled the PE ~1.4us at every quarter boundary)
            o_ps = [ops.tile([128, NCH // 4], f32, tag="ops", name=f"ops{q}")
                    for q in range(4)]

            def dummy(n):
                for _ in range(n):
                    nc.tensor.matmul(
                        d_ps[:], dum[:, 0:1], dum[:], start=True, stop=True)

            def fold(pt, first, last):
                """w_ps[:, nt] += A-block(pt, nt) @ xsum_pt; c_ps += qv_pt."""
                for nt in range(PT):
                    nc.tensor.matmul(
                        w_ps[:, nt:nt + 1],
                        at_sb[:, pt * N + nt * 128: pt * N + (nt + 1) * 128],
                        xm_mm[pt][:], start=False, stop=(last and nt == PT - 1),
                        skip_group_check=True)
                nc.tensor.matmul(
                    c_ps[:], qv_sb[:, pt:pt + 1], xm_mm[pt][:],
                    start=first, stop=last)

            # Accumulations open with single full-region matmuls (coarse
            # PSUM zero-region: per-column start=True flags would lazily
            # wipe earlier columns); u*SA enters w_ps in its open. The
            # opens are emitted just before their first consumer so the
            # dummy stream isn't stalled behind their id8/u8 DMA waits
            # (that stall also reset the PE p-state).
            # (A GpSimd pairwise-add lane + completion-ordered folds were
            # tried: the gp add runs at ~1.9 ns/out-col and its DVE
            # finish gets no 2x — net loss, plus the larger program put
            # two instruction fetches inside pass 2.)
            for pt in range(PT):
                alloc_parts(pt, 4 if pt == PT - 1 else 2)
                if pt < PT - 1:
                    emit_act_dve(pt, 0, L, 0)
                    emit_combine(pt, 2)
                else:
                    emit_act_dve(pt, 0, 3 * L // 4, 0)
                    emit_act_dve(pt, 3 * L // 4, L // 4, 2)
                    emit_combine(pt, 4)
                dummy(DUM[mode][pt])
                if pt == 0:
                    nc.tensor.matmul(
                        w_ps[:], id_sb[:], u_sb[:], start=True, stop=False,
                        skip_group_check=True)
                if pt == 4:
                    for q in range(4):
                        nc.tensor.matmul(
                            o_ps[q][:], id_sb[:], zz[:, 0:NCH // 4],
                            start=True, stop=False, skip_group_check=True)
                fold(pt, first=(pt == 0), last=(pt == PT - 1))

            # ---- finalize w8 / c ----
            w_q = spool.tile([128, PT], dtx, tag="wq")
            # w_ps holds w*SA/2 (xsum shipped halved); requantize to w*SW
            hscale = 2.0 if mode == "fp8" else 1.0
            nc.scalar.activation(
                w_q[:], w_ps[:], mybir.ActivationFunctionType.Copy,
                bias=0.0, scale=float(hscale / sa_over_sw))
            c_sb = spool.tile([1, 1], f32, tag="csb")
            nc.vector.tensor_add(c_sb[:], c_ps[:], c0_sb[:])
            # broadcast c*SW across partitions (fp32 matmul: ones^T @ c),
            # concurrent with pass 2 — only the epilogue consumes it
            c128_ps = cps.tile([128, 1], f32, tag="c128p")
            nc.tensor.matmul(
                c128_ps[:], ones_sb[:], c_sb[:], start=True, stop=True)
            c128 = spool.tile([128, 1], f32, tag="c128")
            nc.vector.tensor_copy(c128[:], c128_ps[:])

            # ---- pass 2 (flipped): out[ch*128+p] via x-stationary ----
            # dual-fp8 (KERNEL_DROW=1): x-PAIR stationary [128,2,128]
            # contracts 256 features per matmul, halving the dispatch-
            # bound instruction count (34ns per ldweights+matmul pair).
            drow = (mode == "fp8" and
                    os.environ.get("KERNEL_DROW", "0") == "1")
            xv8 = xall[:].rearrange("p (t l) -> p t l", t=PT)
            out_sb = cpool.tile([128, NCH], f32, tag="osb")
            h = NCH // 4
            for ch in range(NCH):
                q, qc = divmod(ch, h)
                for nt in range(PT):
                    nc.tensor.matmul(
                        o_ps[q][:, qc:qc + 1],
                        xall[:, nt * L + ch * 128: nt * L + (ch + 1) * 128],
                        w_q[:, nt:nt + 1],
                        start=False,
                        stop=(ch == NCH - 1 and nt == PT - 1),
                        skip_group_check=True)
                if qc == h - 1:
                    # epilogue + store per quarter: earlier quarters
                    # overlap the remaining matmuls; only the last 2KB
                    # store sits on the tail
                    sl = slice(ch + 1 - h, ch + 1)
                    nc.vector.tensor_scalar(
                        out=out_sb[:, sl], in0=o_ps[q][:],
                        scalar1=c128[:, 0:1], scalar2=1.0 / SW,
                        op0=mybir.AluOpType.add, op1=mybir.AluOpType.mult)
                    nc.sync.dma_start(out_d[:, sl], out_sb[:, sl])

    nc.compile()
    return nc


def _host_consts(inputs, mode: str):
    import ml_dtypes

    Wq = np.asarray(inputs["Wq"], np.float64)
    bq = np.asarray(inputs["bq"], np.float64)
    Wk = np.asarray(inputs["Wk"], np.float64)
    bk = np.asarray(inputs["bk"], np.float64)
    Wfc = np.asarray(inputs["Wfc"], np.float64)
    bfc = np.asarray(inputs["bfc"], np.float64)

    s = np.repeat(Wfc[0], D_K) / np.sqrt(D_K)
    A = (Wk * s[:, None]).T @ Wq / L          # [n, p]
    u = Wk.T @ (s * bq)                       # [n]
    qv = Wq.T @ (s * bk) / L                  # [p]
    c0 = float((s * bk) @ bq + bfc[0])

    if mode == "fp8":
        SA = SW * 2.0 ** np.floor(np.log2(448.0 / np.abs(A * SW).max() / 8.0))
    else:
        SA = SW
    return A, u, qv, c0, SA


def _prep_host(inputs, mode: str = MODE):
    """Fold weights on host (f64 accumulate) and lay out per-core arrays."""
    import ml_dtypes

    A, u, qv, c0, SA = _host_consts(inputs, mode)
    bf = ml_dtypes.bfloat16
    np_dtx = ml_dtypes.float8_e4m3fn if mode == "fp8" else bf

    at = np.ascontiguousarray((A * SA).T)     # [p, n], pre-scaled
    atr = np.ascontiguousarray(
        at.reshape(PT, 128, N).transpose(1, 0, 2).reshape(128, PT * N)
    ).astype(np_dtx)
    # fp8: xsum ships halved (e4m3 max 240) -> qv carries 2x; u joins
    # the fold PSUM at the matching SA/2 scale
    uh = 0.5 if mode == "fp8" else 1.0
    qv8 = np.ascontiguousarray(
        (qv * SW / uh).reshape(PT, 128).T).astype(np_dtx)
    u8 = np.ascontiguousarray((u * SA * uh).reshape(PT, 128).T).astype(bf)
    id8 = np.eye(128, dtype=np.float32).astype(bf)
    c0a = np.full((1, 1), c0 * SW, np.float32)
    ones_r = np.ones((1, 128), np.float32)

    x = np.asarray(inputs["x"])
    shared = {"atr": atr, "qv8": qv8, "u8": u8, "id8": id8, "c0": c0a,
              "ones_r": ones_r}
    in_maps = []
    for c in range(NCORES):
        m = dict(shared)
        m["xT"] = np.ascontiguousarray(x[c].T).astype(np_dtx)
        in_maps.append(m)
    return in_maps, SA


LAST_RESULTS = None


def kernel(**inputs) -> np.ndarray:
    global LAST_RESULTS
    _ensure_path()
    from concourse.bass_utils import run_bass_kernel_spmd

    in_maps, SA = _prep_host(inputs, MODE)
    nc = _build(MODE, float(SA / SW))
    kw = {}
    if os.environ.get("KERNEL_TRACE"):
        kw["trace"] = True
    res = run_bass_kernel_spmd(nc, in_maps, list(range(NCORES)), **kw)
    LAST_RESULTS = res
    out = np.stack([
        res.results[c]["out"].T.reshape(L, 1) for c in range(NCORES)
    ])
    return out.astype(np.float32)


if __name__ == "__main__":
    rng = np.random.default_rng(0)
    demo = {
        "x": rng.standard_normal((B, L, N), np.float32),
        "Wq": rng.standard_normal((N, N), np.float32) * 0.03,
        "bq": rng.standard_normal((N,), np.float32) * 0.03,
        "Wk": rng.standard_normal((N, N), np.float32) * 0.03,
        "bk": rng.standard_normal((N,), np.float32) * 0.03,
        "Wfc": rng.standard_normal((1, 16), np.float32) * 0.25,
        "bfc": rng.standard_normal((1,), np.float32) * 0.25,
    }
    o = kernel(**demo)
    print("out", o.shape, o.dtype, float(np.abs(o).max()))


# revision 67
# speedup vs baseline: 1.0540x; 1.0540x over previous
"""Trainium2 Bass kernel for nn_MultiHeadAttention_8684423872640.

Math: the reference collapses algebraically. With
  s[m]   = Wfc[0, m // 64] / sqrt(64)
  A      = (Wk * s[:,None]).T @ Wq / L            # [1024, 1024] weights-only
  u      = Wk.T @ (s * bq)                        # [1024]
  qv     = Wq.T @ (s * bk) / L                    # [1024]
  c0     = (s * bk) @ bq + bfc[0]
the output for batch b is
  xsum_b = sum_l x[b, l, :]                       # [1024]
  w_eff  = A @ xsum_b + u                         # [1024]
  c      = qv @ xsum_b + c0
  out[b, l, 0] = x[b, l, :] @ w_eff + c

Sharding: data-parallel over B — core c handles batch c.

Schedule (fp8 default):
  - x ships as e4m3 (4 MiB/core) and A as e4m3 at a static pow2 scale
    (1 MiB): 5.3 MB total vs 10.5 MB for bf16 — the DMA-engine pool
    (~360 GB/s, serialized) is the floor, so bytes are wall-clock.
    Single SP HWDGE ring, A slices interleaved between x tiles, tail x
    tile in two halves to shorten the last dependency chain.
  - rowsums: each tile split ACT (activation-copy accum) / DVE
    (tensor_reduce) by engine rate; xsum lands ~1.5-2us after its tile.
  - folds: PSUM [128,8] accumulation opened by ONE identity@u matmul
    (start=True covering the whole region — per-column start flags
    lazily re-mark the coarse PSUM zero-region and wipe earlier
    accumulation), then A-block@xsum 1-col matmuls, ldweights hidden
    (~25ns each). Dummy 512-col matmuls pad the PE queue so the DVFS
    p-state stays at 2.4GHz (idle drops it to 1.2GHz for ~3us).
  - pass 2 flipped: x chunks are the STATIONARY ([128,128], loads
    hidden), w8=e4m3(w*2^13) the moving 1-col operand: 256 matmuls
    ~26ns each ≈ 6.7us, accumulating out as PSUM [128, 32] (l = ch*128
    + p), opened early by one zeroing matmul. One DVE epilogue
    (o + c*SW)/SW and one 16KB store.
"""

import os
import sys
import functools
import numpy as np

B, L, N = 8, 4096, 1024
D_K = 64
NCORES = 8
PT = N // 128   # 8 feature tiles
NCH = L // 128  # 32 pass-2 output chunks

SW = 2.0 ** 13          # w / qv / c scale so fp8(w*SW) is in normal range

_TRN_REPO = "/opt/trn_rl_repo"

MODE = os.environ.get("KERNEL_MODE", "fp8")   # "fp8" | "bf16"
# A-matrix extra scale (pow2): at8 = fp8(A * SA); w dequant folds SW/SA
# into the quantize-activation's scale. Chosen so fp8(A*SA) has ~8x
# headroom under the 448 e4m3 max.
SA_EXTRA = {"fp8": None, "bf16": 1.0}

DUM = {
    "fp8": [17, 11, 11, 22, 8, 7, 6, 3],
    "bf16": [24, 8, 13, 13, 13, 13, 13, 11],
}


def _ensure_path():
    if _TRN_REPO not in sys.path and os.path.isdir(_TRN_REPO):
        sys.path.insert(0, _TRN_REPO)


@functools.lru_cache(maxsize=2)
def _build(mode: str, sa_over_sw: float):
    """Build + compile the per-core Bass program. sa_over_sw = SA/SW."""
    _ensure_path()
    import concourse.bass as bass  # noqa: F401
    import concourse.tile as tile
    from concourse import bacc, mybir

    f32 = mybir.dt.float32
    bf16 = mybir.dt.bfloat16
    fp8 = mybir.dt.float8e4
    dtx = fp8 if mode == "fp8" else bf16

    nc = bacc.Bacc(
        "TRN2",
        target_bir_lowering=False,
        debug=False,
        enable_asserts=False,
        num_devices=NCORES,
    )

    xT = nc.dram_tensor("xT", [N, L], dtx, kind="ExternalInput").ap()
    atr = nc.dram_tensor("atr", [128, PT * N], dtx, kind="ExternalInput").ap()
    qv8 = nc.dram_tensor("qv8", [128, PT], dtx, kind="ExternalInput").ap()
    u8 = nc.dram_tensor("u8", [128, PT], bf16, kind="ExternalInput").ap()
    id8 = nc.dram_tensor("id8", [128, 128], bf16, kind="ExternalInput").ap()
    ones_r = nc.dram_tensor("ones_r", [1, 128], f32, kind="ExternalInput").ap()
    c0 = nc.dram_tensor("c0", [1, 1], f32, kind="ExternalInput").ap()
    out_d = nc.dram_tensor("out", [128, NCH], f32, kind="ExternalOutput").ap()

    with tile.TileContext(nc) as tc:
        with (
            tc.tile_pool(name="xpool", bufs=1) as xpool,
            tc.tile_pool(name="cpool", bufs=1) as cpool,
            tc.tile_pool(name="spool", bufs=2) as spool,
            tc.tile_pool(name="xsums", bufs=18) as xsums,
            tc.tile_pool(name="wps", bufs=1, space="PSUM") as wps,
            tc.tile_pool(name="cps", bufs=1, space="PSUM") as cps,
            tc.tile_pool(name="ops", bufs=1, space="PSUM") as ops,
            tc.tile_pool(name="dps", bufs=1, space="PSUM") as dps,
        ):
            # ---- small constants: memsets + DVE-ring DMAs first so the
            # PE's opening matmuls and dummies can start right after the
            # preamble (SWDGE issue is ~1us per DMA — too slow for these)
            dum = cpool.tile([128, 512], dtx, tag="dum")
            nc.gpsimd.memset(dum[:], 1.0)
            zz = cpool.tile([128, NCH], bf16, tag="zz")
            nc.gpsimd.memset(zz[:], 0.0)
            id_sb = cpool.tile([128, 128], bf16, tag="id")
            nc.scalar.dma_start(id_sb[:], id8[:])
            u_sb = cpool.tile([128, PT], bf16, tag="u")
            nc.scalar.dma_start(u_sb[:], u8[:])
            # qv/c0/ones ride the ACT ring too: SWDGE issues are ~1us
            # each and their packets land mid-x-stream in the shared DMA
            # pool; ACT's first rowsum is x0-gated until ~11us anyway,
            # and this frees gpsimd for the combine ops from the start.
            qv_sb = cpool.tile([128, PT], dtx, tag="qv")
            nc.scalar.dma_start(qv_sb[:], qv8[:])
            c0_sb = cpool.tile([1, 1], f32, tag="c0")
            nc.scalar.dma_start(c0_sb[:], c0[:])
            ones_sb = cpool.tile([1, 128], f32, tag="ones")
            nc.scalar.dma_start(ones_sb[:], ones_r[:])

            # ---- bulk stream: one SP HWDGE ring, explicit order ----
            xall = xpool.tile([128, PT * L], dtx, tag="x")
            at_sb = cpool.tile([128, PT * N], dtx, tag="at")

            xv3 = xall[:].rearrange("p (t l) -> p t l", t=PT)

            def dma_x(pt, lo=0, hi=L):
                nc.sync.dma_start(
                    xall[:, pt * L + lo: pt * L + hi],
                    xT[pt * 128:(pt + 1) * 128, lo:hi])

            def dma_x_pair(pt):
                # two row-tiles of xT in ONE dma (fewer SEQ issues: the
                # sync queue's ~0.65us/issue was outpacing the transfers);
                # src rearranged so dims pair 1:1 with the dst view
                src = xT[pt * 128:(pt + 2) * 128, :].rearrange(
                    "(t p) l -> p t l", t=2)
                nc.sync.dma_start(xv3[:, pt:pt + 2, :], src)

            def dma_at2(pt):
                nc.sync.dma_start(
                    at_sb[:, pt * N:(pt + 2) * N], atr[:, pt * N:(pt + 2) * N])

            # x0 in halves: ACT's tile-0 slice is exactly [0:2048], so it
            # starts ~1us earlier (byte-range tracking gates each engine
            # on just its half); DVE's slice is the second half.
            dma_x(0, 0, L // 2)
            dma_x(0, L // 2, L)
            dma_x(1)
            dma_at2(0)
            dma_x(2)
            dma_x(3)
            dma_at2(2)
            dma_x(4)
            dma_x(5)
            dma_at2(4)
            dma_x(6)
            dma_at2(6)
            dma_x(7, 0, 7 * L // 8)
            dma_x(7, 7 * L // 8, L)

            # ---- rowsums: ACT / DVE / GPSIMD 3-lane split per tile ----
            # fp8 rates: ACT .97 ns/col (+278ns accum read), DVE 1.12
            # (2x needs 2-byte dtypes). GPSIMD can't X-reduce but CAN
            # pairwise-add fp8 halves into bf16 (~2 ns/out-col), which
            # DVE then reduces at its 2x bf16 rate — a third lane that
            # turns the 2-engine backlog into slack.
            act_scr = cpool.tile([128, 2304], dtx, tag="ascr")

            xm_mm = [None] * PT   # fold + c-path moving operand

            # lane budget: ACT .97 ns/col + 278ns accum read; DVE 1.12
            # ns/col (fp8 is 1x; 2x needs 2-byte dtypes) + combine. The
            # serial ACT+DVE work (~19us each for all 8 tiles) exceeds
            # the 15us stream, so GPSIMD takes tiles 2 and 5 entirely
            # (pairwise fp8 add at ~2.4 ns/out-col, slow but free
            # capacity) and DVE finishes them at its 2x bf16 rate.
            ACT_SHARE = (16, 32) if mode == "fp8" else (10, 32)
            gp_scr = cpool.tile([128, L], bf16, tag="gscr")
            partmap = {}

            def emit_act_dve(pt, lo, w, base):
                src = lambda a, b: xall[:, pt * L + lo + a: pt * L + lo + b]
                wa = w * ACT_SHARE[0] // ACT_SHARE[1] // 16 * 16
                parts = partmap[pt]
                nc.scalar.activation(
                    act_scr[:, 0:wa], src(0, wa),
                    mybir.ActivationFunctionType.Copy,
                    bias=0.0, accum_out=parts[:, base:base + 1])
                nc.vector.tensor_reduce(
                    parts[:, base + 1:base + 2], src(wa, w),
                    axis=mybir.AxisListType.X, op=mybir.AluOpType.add)

            def emit_gp_add(pt, slot):
                nc.gpsimd.tensor_add(
                    gp_scr[:, slot * (L // 2):slot * (L // 2) + L // 2],
                    xall[:, pt * L: pt * L + L // 2],
                    xall[:, pt * L + L // 2: (pt + 1) * L])

            def emit_gp_reduce(pt, slot):
                nc.vector.tensor_reduce(
                    partmap[pt][:, 0:1],
                    gp_scr[:, slot * (L // 2):slot * (L // 2) + L // 2],
                    axis=mybir.AxisListType.X, op=mybir.AluOpType.add)

            def emit_combine(pt, n):
                # fp8: combine + convert ride the otherwise-idle GPSIMD
                # (tiny [128,1] ops) so DVE's serial rowsum chain stays
                # pure reduces. xsum ships halved: device fp8e4 is e4m3
                # max 240 (not e4m3fn's 448) and |xsum| reaches ~430;
                # the w dequant and host-side qv scale compensate.
                parts = partmap[pt]
                if mode == "fp8":
                    xs = xsums.tile([128, 1], f32, tag="xs", name=f"xs{pt}")
                    nc.gpsimd.tensor_add(xs[:], parts[:, 0:1], parts[:, 1:2])
                    for j in range(2, n):
                        nc.gpsimd.tensor_add(xs[:], xs[:], parts[:, j:j + 1])
                    xm_mm[pt] = xsums.tile([128, 1], fp8, tag="x8", name=f"x8{pt}")
                    nc.gpsimd.tensor_scalar_mul(xm_mm[pt][:], xs[:], 0.5)
                else:
                    xs = xsums.tile([128, 1], f32, tag="xs", name=f"xs{pt}")
                    nc.vector.tensor_reduce(
                        xs[:], parts[:, 0:n], axis=mybir.AxisListType.X,
                        op=mybir.AluOpType.add)
                    xm_mm[pt] = xsums.tile([128, 1], bf16, tag="xb", name=f"xb{pt}")
                    nc.vector.tensor_copy(xm_mm[pt][:], xs[:])

            def alloc_parts(pt, n):
                partmap[pt] = xsums.tile([128, n], f32, tag="p", name=f"p{pt}")

            # ---- PE stream ----
            w_ps = wps.tile([128, PT], f32, tag="wps")
            c_ps = cps.tile([1, 1], f32, tag="cps")
            d_ps = dps.tile([1, 512], f32, tag="dps")
            o_ps = ops.tile([128, NCH], f32, tag="ops")

            def dummy(n):
                for _ in range(n):
                    nc.tensor.matmul(
                        d_ps[:], dum[:, 0:1], dum[:], start=True, stop=True)

            def fold(pt, first, last):
                """w_ps[:, nt] += A-block(pt, nt) @ xsum_pt; c_ps += qv_pt."""
                for nt in range(PT):
                    nc.tensor.matmul(
                        w_ps[:, nt:nt + 1],
                        at_sb[:, pt * N + nt * 128: pt * N + (nt + 1) * 128],
                        xm_mm[pt][:], start=False, stop=(last and nt == PT - 1),
                        skip_group_check=True)
                nc.tensor.matmul(
                    c_ps[:], qv_sb[:, pt:pt + 1], xm_mm[pt][:],
                    start=first, stop=last)

            # Accumulations open with single full-region matmuls (coarse
            # PSUM zero-region: per-column start=True flags would lazily
            # wipe earlier columns); u*SA enters w_ps in its open. The
            # opens are emitted just before their first consumer so the
            # dummy stream isn't stalled behind their id8/u8 DMA waits
            # (that stall also reset the PE p-state).
            # (A GpSimd pairwise-add lane + completion-ordered folds were
            # tried: the gp add runs at ~1.9 ns/out-col and its DVE
            # finish gets no 2x — net loss, plus the larger program put
            # two instruction fetches inside pass 2.)
            for pt in range(PT):
                alloc_parts(pt, 4 if pt == PT - 1 else 2)
                if pt < PT - 1:
                    emit_act_dve(pt, 0, L, 0)
                    emit_combine(pt, 2)
                else:
                    emit_act_dve(pt, 0, 7 * L // 8, 0)
                    emit_act_dve(pt, 7 * L // 8, L // 8, 2)
                    emit_combine(pt, 4)
                dummy(DUM[mode][pt])
                if pt == 0:
                    nc.tensor.matmul(
                        w_ps[:], id_sb[:], u_sb[:], start=True, stop=False,
                        skip_group_check=True)
                if pt == 4:
                    nc.tensor.matmul(
                        o_ps[:], id_sb[:], zz[:], start=True, stop=False,
                        skip_group_check=True)
                fold(pt, first=(pt == 0), last=(pt == PT - 1))

            # ---- finalize w8 / c ----
            w_q = spool.tile([128, PT], dtx, tag="wq")
            # w_ps holds w*SA/2 (xsum shipped halved); requantize to w*SW
            hscale = 2.0 if mode == "fp8" else 1.0
            nc.scalar.activation(
                w_q[:], w_ps[:], mybir.ActivationFunctionType.Copy,
                bias=0.0, scale=float(hscale / sa_over_sw))
            c_sb = spool.tile([1, 1], f32, tag="csb")
            nc.vector.tensor_add(c_sb[:], c_ps[:], c0_sb[:])
            # broadcast c*SW across partitions (fp32 matmul: ones^T @ c),
            # concurrent with pass 2 — only the epilogue consumes it
            c128_ps = cps.tile([128, 1], f32, tag="c128p")
            nc.tensor.matmul(
                c128_ps[:], ones_sb[:], c_sb[:], start=True, stop=True)
            c128 = spool.tile([128, 1], f32, tag="c128")
            nc.vector.tensor_copy(c128[:], c128_ps[:])

            # ---- pass 2 (flipped): out[ch*128+p] via x-stationary ----
            # dual-fp8 (KERNEL_DROW=1): x-PAIR stationary [128,2,128]
            # contracts 256 features per matmul, halving the dispatch-
            # bound instruction count (34ns per ldweights+matmul pair).
            drow = (mode == "fp8" and
                    os.environ.get("KERNEL_DROW", "0") == "1")
            xv8 = xall[:].rearrange("p (t l) -> p t l", t=PT)
            out_sb = cpool.tile([128, NCH], f32, tag="osb")
            for ch in range(NCH):
                for nt in range(PT):
                    nc.tensor.matmul(
                        o_ps[:, ch:ch + 1],
                        xall[:, nt * L + ch * 128: nt * L + (ch + 1) * 128],
                        w_q[:, nt:nt + 1],
                        start=False,
                        stop=(ch == NCH - 1 and nt == PT - 1),
                        skip_group_check=True)
                if (ch + 1) % (NCH // 4) == 0:
                    # epilogue + store per quarter: earlier quarters
                    # overlap the remaining matmuls; only the last 2KB
                    # store sits on the tail
                    h = NCH // 4
                    sl = slice(ch + 1 - h, ch + 1)
                    nc.vector.tensor_scalar(
                        out=out_sb[:, sl], in0=o_ps[:, sl],
                        scalar1=c128[:, 0:1], scalar2=1.0 / SW,
                        op0=mybir.AluOpType.add, op1=mybir.AluOpType.mult)
                    nc.sync.dma_start(out_d[:, sl], out_sb[:, sl])

    nc.compile()
    return nc


def _host_consts(inputs, mode: str):
    import ml_dtypes

    Wq = np.asarray(inputs["Wq"], np.float64)
    bq = np.asarray(inputs["bq"], np.float64)
    Wk = np.asarray(inputs["Wk"], np.float64)
    bk = np.asarray(inputs["bk"], np.float64)
    Wfc = np.asarray(inputs["Wfc"], np.float64)
    bfc = np.asarray(inputs["bfc"], np.float64)

    s = np.repeat(Wfc[0], D_K) / np.sqrt(D_K)
    A = (Wk * s[:, None]).T @ Wq / L          # [n, p]
    u = Wk.T @ (s * bq)                       # [n]
    qv = Wq.T @ (s * bk) / L                  # [p]
    c0 = float((s * bk) @ bq + bfc[0])

    if mode == "fp8":
        SA = SW * 2.0 ** np.floor(np.log2(448.0 / np.abs(A * SW).max() / 8.0))
    else:
        SA = SW
    return A, u, qv, c0, SA


def _prep_host(inputs, mode: str = MODE):
    """Fold weights on host (f64 accumulate) and lay out per-core arrays."""
    import ml_dtypes

    A, u, qv, c0, SA = _host_consts(inputs, mode)
    bf = ml_dtypes.bfloat16
    np_dtx = ml_dtypes.float8_e4m3fn if mode == "fp8" else bf

    at = np.ascontiguousarray((A * SA).T)     # [p, n], pre-scaled
    atr = np.ascontiguousarray(
        at.reshape(PT, 128, N).transpose(1, 0, 2).reshape(128, PT * N)
    ).astype(np_dtx)
    # fp8: xsum ships halved (e4m3 max 240) -> qv carries 2x; u joins
    # the fold PSUM at the matching SA/2 scale
    uh = 0.5 if mode == "fp8" else 1.0
    qv8 = np.ascontiguousarray(
        (qv * SW / uh).reshape(PT, 128).T).astype(np_dtx)
    u8 = np.ascontiguousarray((u * SA * uh).reshape(PT, 128).T).astype(bf)
    id8 = np.eye(128, dtype=np.float32).astype(bf)
    c0a = np.full((1, 1), c0 * SW, np.float32)
    ones_r = np.ones((1, 128), np.float32)

    x = np.asarray(inputs["x"])
    shared = {"atr": atr, "qv8": qv8, "u8": u8, "id8": id8, "c0": c0a,
              "ones_r": ones_r}
    in_maps = []
    for c in range(NCORES):
        m = dict(shared)
        m["xT"] = np.ascontiguousarray(x[c].T).astype(np_dtx)
        in_maps.append(m)
    return in_maps, SA


LAST_RESULTS = None


def kernel(**inputs) -> np.ndarray:
    global LAST_RESULTS
    _ensure_path()
    from concourse.bass_utils import run_bass_kernel_spmd

    in_maps, SA = _prep_host(inputs, MODE)
    nc = _build(MODE, float(SA / SW))
    kw = {}
    if os.environ.get("KERNEL_TRACE"):
        kw["trace"] = True
    res = run_bass_kernel_spmd(nc, in_maps, list(range(NCORES)), **kw)
    LAST_RESULTS = res
    out = np.stack([
        res.results[c]["out"].T.reshape(L, 1) for c in range(NCORES)
    ])
    return out.astype(np.float32)


if __name__ == "__main__":
    rng = np.random.default_rng(0)
    demo = {
        "x": rng.standard_normal((B, L, N), np.float32),
        "Wq": rng.standard_normal((N, N), np.float32) * 0.03,
        "bq": rng.standard_normal((N,), np.float32) * 0.03,
        "Wk": rng.standard_normal((N, N), np.float32) * 0.03,
        "bk": rng.standard_normal((N,), np.float32) * 0.03,
        "Wfc": rng.standard_normal((1, 16), np.float32) * 0.25,
        "bfc": rng.standard_normal((1,), np.float32) * 0.25,
    }
    o = kernel(**demo)
    print("out", o.shape, o.dtype, float(np.abs(o).max()))
